# revision 41
# baseline (speedup 1.0000x reference)
"""HAN (hierarchical attention network) forward pass on 8 TRN2 NeuronCores.

Strategy
--------
Data-parallel over batch: each core handles 8 documents = 128 sentences =
4096 tokens, fully independently (no collectives). Inside a core:

* Embedding lookup + word-GRU input projection folded on host:
  gi = (emb @ Wih.T)[tokens], gathered per step with indirect DMA.
* Word bi-GRU, batch-major [128 sentences, feat]. Per-direction gate math
  (FD=256 elementwise ops) so the two independent direction recurrences
  pipeline across ACT/DVE. PE program order is arranged so that the
  next step's gi injection and the previous step's attention matmuls
  execute while the gate-math chain runs (keeps PE warm, no HAM
  re-throttle).
* Word attention: scores accumulated in-loop (u = tanh(h Wa + ba) lagged
  one step; u.v via stt-accum on GpSimd). Softmax without max-subtraction
  (host-validated score bounds) as an incremental exp-weighted running
  sum, so the end-of-loop softmax valley disappears.
* Sentence bi-GRU: same structure, batch 8. Sentence attention is a
  post-pass batch GEMM over a [128 rows=(step,doc)] layout; the
  per-document softmax is done with indicator matmuls (segmented sum +
  broadcast-back), and the weighted sum is a single matmul with a
  masked-diagonal lhsT.

Compute dtype bf16 (fp32 PSUM accumulation + fp32 attention
accumulators).
"""

import numpy as np
import ml_dtypes

import concourse.bass as bass
import concourse.mybir as mybir
import concourse.tile as tile
from concourse import bacc, bass_utils
from concourse.masks import make_identity

BF = mybir.dt.bfloat16
F32 = mybir.dt.float32
AF = mybir.ActivationFunctionType
ALU = mybir.AluOpType
bf16 = ml_dtypes.bfloat16

V, E = 50000, 300
HW_, HS_ = 256, 256
NCLS = 10
B, S, W = 64, 16, 32
NCORES = 8
BC = B // NCORES          # docs per core = 8
NW = BC * S               # word-level batch per core = 128
GW = 3 * HW_              # 768


def _build_program():
    nc = bacc.Bacc(
        "TRN2",
        target_bir_lowering=False,
        debug=False,
        enable_asserts=False,
        num_devices=NCORES,
    )

    # ---- DRAM I/O ----
    h = {}
    h["G"] = nc.dram_tensor("G", [V, 1536], BF, kind="ExternalInput")
    h["toks"] = nc.dram_tensor("toks", [128, 32], mybir.dt.int32, kind="ExternalInput")
    h["whhT"] = nc.dram_tensor("whhT", [4, 128, GW], BF, kind="ExternalInput")
    h["brow"] = nc.dram_tensor("brow", [1, 512], BF, kind="ExternalInput")
    h["waT"] = nc.dram_tensor("waT", [512, 512], BF, kind="ExternalInput")
    h["barow"] = nc.dram_tensor("barow", [1, 512], BF, kind="ExternalInput")
    h["vb"] = nc.dram_tensor("vb", [128, 512], BF, kind="ExternalInput")
    h["swihT"] = nc.dram_tensor("swihT", [512, 1536], BF, kind="ExternalInput")
    h["sprow"] = nc.dram_tensor("sprow", [1, 1536], BF, kind="ExternalInput")
    h["swhhT"] = nc.dram_tensor("swhhT", [4, 128, GW], BF, kind="ExternalInput")
    h["swhhF"] = nc.dram_tensor("swhhF", [24, 128, 128], BF, kind="ExternalInput")
    h["sbrowF"] = nc.dram_tensor("sbrowF", [4, 128], BF, kind="ExternalInput")
    h["bones"] = nc.dram_tensor("bones", [4, 32], BF, kind="ExternalInput")
    h["sbrow"] = nc.dram_tensor("sbrow", [1, 512], BF, kind="ExternalInput")
    h["sawT"] = nc.dram_tensor("sawT", [512, 512], BF, kind="ExternalInput")
    h["sbarow"] = nc.dram_tensor("sbarow", [1, 512], BF, kind="ExternalInput")
    h["svb"] = nc.dram_tensor("svb", [128, 512], BF, kind="ExternalInput")
    h["ind8"] = nc.dram_tensor("ind8", [128, 8], BF, kind="ExternalInput")
    h["ind8f"] = nc.dram_tensor("ind8f", [128, 8], F32, kind="ExternalInput")
    h["ind8T"] = nc.dram_tensor("ind8T", [8, 128], F32, kind="ExternalInput")
    h["fcwT"] = nc.dram_tensor("fcwT", [512, NCLS], BF, kind="ExternalInput")
    h["fcb"] = nc.dram_tensor("fcb", [1, NCLS], BF, kind="ExternalInput")
    h["out"] = nc.dram_tensor("out", [BC, NCLS], F32, kind="ExternalOutput")

    with tile.TileContext(nc) as tc:
        _body(nc, tc, h)
    nc.compile()
    return nc


def _body(nc, tc, handles):
    def dram(name):
        return handles[name].ap()

    G_ap = dram("G")
    with tc.tile_pool(name="const", bufs=1) as cp:
        # ---- constants / weights in SBUF ----
        ident = cp.tile([128, 128], BF)
        make_identity(nc, ident)
        ones = cp.tile([1, 128], BF)
        nc.gpsimd.memset(ones, 1.0)

        toks = cp.tile([128, 32], mybir.dt.int32)
        nc.sync.dma_start(out=toks, in_=dram("toks"))
        whh = cp.tile([128, 4 * GW], BF)  # (d0k0 d0k1 d1k0 d1k1); [rz(512) n(256)]
        for j in range(4):
            nc.sync.dma_start(out=whh[:, j * GW:(j + 1) * GW],
                              in_=dram("whhT")[j])
        brow = cp.tile([1, 512], BF)
        nc.sync.dma_start(out=brow, in_=dram("brow"))
        waT = cp.tile([128, 4 * 512], BF)
        for j in range(4):
            nc.sync.dma_start(out=waT[:, j * 512:(j + 1) * 512],
                              in_=dram("waT")[j * 128:(j + 1) * 128, :])
        barow = cp.tile([1, 512], BF)
        nc.sync.dma_start(out=barow, in_=dram("barow"))
        vb = cp.tile([128, 512], BF)
        nc.sync.dma_start(out=vb, in_=dram("vb"))

        swihT = cp.tile([128, 4 * 1536], BF)
        for j in range(4):
            nc.sync.dma_start(out=swihT[:, j * 1536:(j + 1) * 1536],
                              in_=dram("swihT")[j * 128:(j + 1) * 128, :])
        sprow = cp.tile([1, 1536], BF)
        nc.sync.dma_start(out=sprow, in_=dram("sprow"))
        swhhF = cp.tile([128, 24 * 128], BF)
        for j in range(24):
            nc.sync.dma_start(out=swhhF[:, j * 128:(j + 1) * 128],
                              in_=dram("swhhF")[j])
        sbrow = cp.tile([1, 512], BF)
        nc.sync.dma_start(out=sbrow, in_=dram("sbrow"))
        sbrowF = cp.tile([4, 128], BF)
        nc.sync.dma_start(out=sbrowF, in_=dram("sbrowF"))
        bones = cp.tile([4, 32], BF)
        nc.sync.dma_start(out=bones, in_=dram("bones"))
        sawT = cp.tile([128, 4 * 512], BF)
        for j in range(4):
            nc.sync.dma_start(out=sawT[:, j * 512:(j + 1) * 512],
                              in_=dram("sawT")[j * 128:(j + 1) * 128, :])
        sbarow = cp.tile([1, 512], BF)
        nc.sync.dma_start(out=sbarow, in_=dram("sbarow"))
        svb = cp.tile([128, 512], BF)
        nc.sync.dma_start(out=svb, in_=dram("svb"))
        ind8 = cp.tile([128, 8], BF)
        nc.sync.dma_start(out=ind8, in_=dram("ind8"))
        ind8f = cp.tile([128, 8], F32)
        nc.sync.dma_start(out=ind8f, in_=dram("ind8f"))
        ind8T = cp.tile([8, 128], F32)
        nc.sync.dma_start(out=ind8T, in_=dram("ind8T"))
        fcwT = cp.tile([128, 4 * NCLS], BF)
        for j in range(4):
            nc.sync.dma_start(out=fcwT[:, j * NCLS:(j + 1) * NCLS],
                              in_=dram("fcwT")[j * 128:(j + 1) * 128, :])
        fcb = cp.tile([1, NCLS], BF)
        nc.sync.dma_start(out=fcb, in_=dram("fcb"))

        # ---- persistent state ----
        hw_hist = cp.tile([128, 33 * 512], BF)   # h_t history, slot 0 = zeros
        nc.gpsimd.memset(hw_hist[:, 0:512], 0.0)
        hT0 = cp.tile([128, 512], BF)            # transposed h state, step -1
        nc.gpsimd.memset(hT0, 0.0)
        scores = cp.tile([128, 32], F32)
        ew = cp.tile([128, 32], F32)             # exp(scores)
        separts = cp.tile([128, 8], F32)         # partial exp sums (per 4-batch)
        wacc = cp.tile([128, 512], F32)          # running exp-weighted h sum
        nc.gpsimd.memset(wacc, 0.0)
        sent = cp.tile([128, 512], BF)           # word-attention output
        sgi = cp.tile([128, 1536], BF)           # sentence-GRU input projections
        psgT = cp.tile([128, 12 * 128], BF)      # transposed gi: gate-chunk x rows
        hsf_hist = cp.tile([128, 17 * 32], BF)   # feature-major h^T history
        nc.gpsimd.memset(hsf_hist[:, 0:32], 0.0)
        hsb = cp.tile([128, 512], BF)            # sentence h, batch rows (s,d)
        hsbT = cp.tile([128, 4 * 128], BF)       # transposed: feat-chunk x rows

        # ================= word stage =================
        with tc.tile_pool(name="wp", bufs=3) as wp, \
             tc.tile_pool(name="wgi", bufs=5) as wgi, \
             tc.tile_pool(name="pg", bufs=2, space="PSUM") as pgp, \
             tc.tile_pool(name="pn2", bufs=1, space="PSUM") as pnp, \
             tc.tile_pool(name="pwa", bufs=1, space="PSUM") as pwp, \
             tc.tile_pool(name="pt", bufs=1, space="PSUM") as ptp, \
             tc.tile_pool(name="pu", bufs=1, space="PSUM") as pup:

            PRE = 3  # gather prefetch depth
            gi_tiles = {}
            for t in range(PRE):
                g = wgi.tile([128, 1536], BF, tag="gi")
                nc.gpsimd.indirect_dma_start(
                    out=g[:, :], out_offset=None, in_=G_ap[:, :],
                    in_offset=bass.IndirectOffsetOnAxis(ap=toks[:, t:t + 1], axis=0),
                )
                gi_tiles[t] = g

            def inject(t):
                """Open PSUM accumulation groups for step t with gi + biases."""
                gi = gi_tiles[t]
                a = pgp.tile([128, 512], F32, tag="pgd0")
                b = pgp.tile([128, 512], F32, tag="pgd1")
                p = pnp.tile([128, 512], F32, tag="pn")
                nc.tensor.matmul(a, lhsT=ident, rhs=gi[:, 0:512],
                                 start=True, stop=False)
                nc.tensor.matmul(b, lhsT=ident, rhs=gi[:, 512:1024],
                                 start=True, stop=False)
                nc.tensor.matmul(p, lhsT=ones, rhs=brow,
                                 start=True, stop=False)
                return a, b, p

            pg_next = inject(0)
            pwa = pwp.tile([128, 512], F32, tag="pwa")

            prev_hT = hT0
            prev_u = None
            wsum_p = 0  # next pending weighted-sum step

            for t in range(32):
                pga, pgb, pn = pg_next
                gi = gi_tiles.pop(t)

                # --- PE: recurrent matmuls for step t (need prev_hT) ---
                for dd in range(2):
                    tgt = pga if dd == 0 else pgb
                    for k in range(2):
                        lhs = prev_hT[:, (dd * 2 + k) * 128:(dd * 2 + k + 1) * 128]
                        w = whh[:, (dd * 2 + k) * GW:(dd * 2 + k + 1) * GW]
                        nc.tensor.matmul(tgt, lhsT=lhs, rhs=w[:, 0:512],
                                         start=False, stop=(k == 1))
                for dd in range(2):
                    for k in range(2):
                        lhs = prev_hT[:, (dd * 2 + k) * 128:(dd * 2 + k + 1) * 128]
                        w = whh[:, (dd * 2 + k) * GW:(dd * 2 + k + 1) * GW]
                        nc.tensor.matmul(pn[:, dd * 256:(dd + 1) * 256],
                                         lhsT=lhs, rhs=w[:, 512:768],
                                         start=False,
                                         stop=(dd == 1 and k == 1))

                # --- PE: attention matmuls for step t-1 (fills stall) ---
                if t >= 1:
                    pu = pup.tile([128, 512], F32, tag="pu")
                    nc.tensor.matmul(pu, lhsT=ones, rhs=barow,
                                     start=True, stop=False)
                    for j in range(4):
                        nc.tensor.matmul(pu, lhsT=prev_hT[:, j * 128:(j + 1) * 128],
                                         rhs=waT[:, j * 512:(j + 1) * 512],
                                         start=False, stop=(j == 3))

                # --- PE: inject step t+1 (gi ready from DMA) ---
                if t < 31:
                    pg_next = inject(t + 1)

                # --- DMA: prefetch gather for step t+PRE ---
                if t + PRE < 32:
                    g = wgi.tile([128, 1536], BF, tag="gi")
                    nc.gpsimd.indirect_dma_start(
                        out=g[:, :], out_offset=None, in_=G_ap[:, :],
                        in_offset=bass.IndirectOffsetOnAxis(
                            ap=toks[:, t + PRE:t + PRE + 1], axis=0),
                    )
                    gi_tiles[t + PRE] = g

                # --- gate math, per direction (ACT/DVE pipeline) ---
                # h' = (1-z).n + z.h_prev with (1-z) on DVE and z.h_prev on
                # GpSimd, both off the critical chain.
                h_prev = hw_hist[:, t * 512:(t + 1) * 512]
                h_new = hw_hist[:, (t + 1) * 512:(t + 2) * 512]
                rz = wp.tile([128, 1024], BF, tag="rz")  # [r0 z0 r1 z1]
                nn = wp.tile([128, 512], BF, tag="nn")   # [n0 n1]
                for dd in range(2):
                    pgd = pga if dd == 0 else pgb
                    rzd = rz[:, dd * 512:(dd + 1) * 512]
                    nc.scalar.activation(rzd, pgd, AF.Sigmoid)
                    r_d = rz[:, dd * 512:dd * 512 + 256]
                    z_d = rz[:, dd * 512 + 256:(dd + 1) * 512]
                    t1 = wp.tile([128, 256], BF, tag=f"t1{dd}")
                    nc.vector.tensor_tensor(t1, r_d, pn[:, dd * 256:(dd + 1) * 256],
                                            op=ALU.mult)
                    npre = wp.tile([128, 256], BF, tag=f"np{dd}")
                    nc.vector.tensor_add(npre, t1,
                                         gi[:, 1024 + dd * 256:1024 + (dd + 1) * 256])
                    omz = wp.tile([128, 256], BF, tag=f"om{dd}")
                    nc.vector.tensor_scalar(out=omz, in0=z_d, scalar1=-1.0,
                                            scalar2=1.0, op0=ALU.mult, op1=ALU.add)
                    zh = wp.tile([128, 256], BF, tag=f"zh{dd}")
                    nc.gpsimd.tensor_tensor(
                        zh, z_d, h_prev[:, dd * 256:(dd + 1) * 256], op=ALU.mult)
                    nnd = nn[:, dd * 256:(dd + 1) * 256]
                    nc.scalar.activation(nnd, npre, AF.Tanh)
                    nom = wp.tile([128, 256], BF, tag=f"nm{dd}")
                    nc.vector.tensor_tensor(nom, nnd, omz, op=ALU.mult)
                    nc.vector.tensor_add(h_new[:, dd * 256:(dd + 1) * 256], nom, zh)

                # --- u(t-1) = tanh(pu); score via stt-accum on GpSimd ---
                if t >= 1:
                    u = wp.tile([128, 512], BF, tag="u")
                    nc.scalar.activation(u, pu, AF.Tanh)
                    scr = wp.tile([128, 512], BF, tag="scr")
                    nc.vector.scalar_tensor_tensor(
                        out=scr, in0=u, scalar=1.0, in1=vb,
                        op0=ALU.mult, op1=ALU.mult,
                        accum_out=scores[:, t - 1:t])
                    prev_u = u

                # --- batched exp of scores, every 4 completed steps ---
                # exp(s) = 1/sigmoid(-s) - 1 (stays in the sigmoid table set;
                # a real Exp would force two ACT_TABLE_LOADs per batch)
                done = t  # scores available: 0..t-1 (scr for t-1 just queued)
                if done % 4 == 0 and done > 0:
                    j = done // 4 - 1
                    sl = slice(j * 4, (j + 1) * 4)
                    nc.scalar.activation(separts[:, 0:4], scores[:, sl],
                                         AF.Sigmoid, scale=-1.0)
                    nc.vector.reciprocal(separts[:, 4:8], separts[:, 0:4])
                    nc.vector.tensor_scalar_add(ew[:, sl], separts[:, 4:8], -1.0)
                # --- lagged weighted-sum: diag matmul, PSUM-resident accum ---
                # (runs on PE during the gate-math stall; no DVE merges)
                if wsum_p < (t // 4) * 4:
                    s = wsum_p
                    dg = wp.tile([128, 128], BF, tag="dg")
                    nc.vector.tensor_scalar_mul(dg, ident, ew[:, s:s + 1])
                    nc.tensor.matmul(pwa, lhsT=dg,
                                     rhs=hw_hist[:, (s + 1) * 512:(s + 2) * 512],
                                     start=(s == 0), stop=(s == 31))
                    wsum_p += 1

                # --- keep-warm dummy matmuls (execute during the stall) ---
                if t >= 1:
                    nc.tensor.matmul(pu[:, 0:512], lhsT=ident,
                                     rhs=waT[:, 512:1024], start=True, stop=True,
                                     skip_group_check=True)
                    nc.tensor.matmul(pu[:, 0:512], lhsT=ident,
                                     rhs=waT[:, 1024:1536], start=True, stop=True,
                                     skip_group_check=True)

                # --- transpose h_new -> hT (stall point, last in PE queue) ---
                pt = ptp.tile([128, 512], BF, tag="pt")
                for j in range(4):
                    nc.tensor.transpose(pt[:, j * 128:(j + 1) * 128],
                                        in_=h_new[:, j * 128:(j + 1) * 128],
                                        identity=ident)
                hT = wp.tile([128, 512], BF, tag="hT")
                nc.vector.tensor_copy(hT[:, 0:256], pt[:, 0:256])
                nc.scalar.copy(hT[:, 256:512], pt[:, 256:512])
                prev_hT = hT

            # ---- word epilogue: attention for t=31 + remaining wsum ----
            pu = pup.tile([128, 512], F32, tag="pu")
            nc.tensor.matmul(pu, lhsT=ones, rhs=barow, start=True, stop=False)
            for j in range(4):
                nc.tensor.matmul(pu, lhsT=prev_hT[:, j * 128:(j + 1) * 128],
                                 rhs=waT[:, j * 512:(j + 1) * 512],
                                 start=False, stop=(j == 3))
            u = wp.tile([128, 512], BF, tag="u")
            nc.scalar.activation(u, pu, AF.Tanh)
            scr = wp.tile([128, 512], BF, tag="scr")
            nc.vector.scalar_tensor_tensor(
                out=scr, in0=u, scalar=1.0, in1=vb,
                op0=ALU.mult, op1=ALU.mult, accum_out=scores[:, 31:32])
            nc.scalar.activation(separts[:, 0:4], scores[:, 28:32],
                                 AF.Sigmoid, scale=-1.0)
            nc.vector.reciprocal(separts[:, 4:8], separts[:, 0:4])
            nc.vector.tensor_scalar_add(ew[:, 28:32], separts[:, 4:8], -1.0)
            # catch up weighted sum (diag matmuls)
            while wsum_p < 32:
                s = wsum_p
                dg = wp.tile([128, 128], BF, tag="dg")
                nc.vector.tensor_scalar_mul(dg, ident, ew[:, s:s + 1])
                nc.tensor.matmul(pwa, lhsT=dg,
                                 rhs=hw_hist[:, (s + 1) * 512:(s + 2) * 512],
                                 start=(s == 0), stop=(s == 31))
                wsum_p += 1
            # normalize: sent = pwa / sum(exp)
            se = wp.tile([128, 1], F32, tag="se")
            nc.vector.tensor_reduce(se, ew, axis=mybir.AxisListType.X,
                                    op=ALU.add)
            rse = wp.tile([128, 1], F32, tag="rse")
            nc.vector.reciprocal(rse, se)
            nc.vector.tensor_scalar_mul(sent, pwa, rse)

        # ---- sent -> sentT + sentence input projections ----
        with tc.tile_pool(name="mid", bufs=1) as mp, \
             tc.tile_pool(name="pmid", bufs=1, space="PSUM") as pmp:
            ptm = pmp.tile([128, 512], BF, tag="ptm")
            for j in range(4):
                nc.tensor.transpose(ptm[:, j * 128:(j + 1) * 128],
                                    in_=sent[:, j * 128:(j + 1) * 128],
                                    identity=ident)
            sentT = mp.tile([128, 512], BF)
            nc.vector.tensor_copy(sentT[:, 0:256], ptm[:, 0:256])
            nc.scalar.copy(sentT[:, 256:512], ptm[:, 256:512])

            psg = pmp.tile([128, 1536], F32, tag="psg")
            for ns in range(3):
                sl = slice(ns * 512, (ns + 1) * 512)
                nc.tensor.matmul(psg[:, sl], lhsT=ones, rhs=sprow[:, sl],
                                 start=True, stop=False)
                for k in range(4):
                    nc.tensor.matmul(psg[:, sl],
                                     lhsT=sentT[:, k * 128:(k + 1) * 128],
                                     rhs=swihT[:, k * 1536 + ns * 512:
                                               k * 1536 + (ns + 1) * 512],
                                     start=False, stop=(k == 3))
            nc.scalar.copy(sgi[:, 0:512], psg[:, 0:512])
            nc.scalar.copy(sgi[:, 512:1024], psg[:, 512:1024])
            nc.vector.tensor_copy(sgi[:, 1024:1536], psg[:, 1024:1536])
            # transpose sgi into feature-major gate-chunk slots:
            # [d0r0 d0r1 d1r0 d1r1 d0z0 d0z1 d1z0 d1z1 d0n0 d0n1 d1n0 d1n1]
            slot_src = [0, 128, 512, 640, 256, 384, 768, 896,
                        1024, 1152, 1280, 1408]
            ptg = pmp.tile([128, 1536], BF, tag="ptg")
            for j, src in enumerate(slot_src):
                nc.tensor.transpose(ptg[:, j * 128:(j + 1) * 128],
                                    in_=sgi[:, src:src + 128],
                                    identity=ident)
            nc.scalar.copy(psgT[:, 0:512], ptg[:, 0:512])
            nc.vector.tensor_copy(psgT[:, 512:1024], ptg[:, 512:1024])
            nc.scalar.copy(psgT[:, 1024:1536], ptg[:, 1024:1536])

        # ================= sentence stage =================
        # Feature-major sentence GRU: state h^T [128 feat(chunk d,m), 8 docs],
        # weight-stationary rec matmuls (N=8), gate math at FD<=64.
        with tc.tile_pool(name="sp", bufs=3) as sp:
          with tc.tile_pool(name="pgs", bufs=2, space="PSUM") as pgsp, \
               tc.tile_pool(name="psd", bufs=1, space="PSUM") as psdp:
            sdum = psdp.tile([128, 512], F32, tag="sdum")
            psgT3 = psgT.rearrange("p (g r) -> p g r", g=12)

            for t in range(16):
                hprev = hsf_hist[:, t * 32:(t + 1) * 32]
                h_new = hsf_hist[:, (t + 1) * 32:(t + 2) * 32]
                prz_full = pgsp.tile([128, 512], F32, tag="prz")
                pnn_full = pgsp.tile([128, 512], F32, tag="pnn")
                prz = prz_full[:, 0:64]
                pnn = pnn_full[:, 0:32]
                # n-gate bias opener: pnn[p, i*8+j] = bhh_n[i*128+p]
                nc.tensor.matmul(pnn, lhsT=sbrowF, rhs=bones,
                                 start=True, stop=False)
                # recurrent: rz slots 0..7, n slots 0..3 (chunk (d, m), K (d, k))
                for s in range(8):
                    d = (s % 4) // 2
                    for k in range(2):
                        w = swhhF[:, (s * 2 + k) * 128:(s * 2 + k + 1) * 128]
                        nc.tensor.matmul(prz[:, s * 8:(s + 1) * 8], lhsT=w,
                                         rhs=hprev[:, (d * 2 + k) * 8:
                                                   (d * 2 + k + 1) * 8],
                                         start=(k == 0), stop=(k == 1))
                for i in range(4):
                    d = i // 2
                    for k in range(2):
                        w = swhhF[:, (16 + i * 2 + k) * 128:
                                  (16 + i * 2 + k + 1) * 128]
                        nc.tensor.matmul(pnn[:, i * 8:(i + 1) * 8], lhsT=w,
                                         rhs=hprev[:, (d * 2 + k) * 8:
                                                   (d * 2 + k + 1) * 8],
                                         start=False,
                                         stop=(i == 3 and k == 1))

                # gate math (tiny FD)
                rzp = sp.tile([128, 64], BF, tag="srzp")
                nc.vector.tensor_tensor(
                    rzp.rearrange("p (g r) -> p g r", g=8),
                    prz.rearrange("p (g r) -> p g r", g=8),
                    psgT3[:, 0:8, t * 8:(t + 1) * 8], op=ALU.add)
                rz = sp.tile([128, 64], BF, tag="srz")
                nc.scalar.activation(rz, rzp, AF.Sigmoid)
                t1 = sp.tile([128, 32], BF, tag="st1")
                nc.vector.tensor_tensor(t1, rz[:, 0:32], pnn, op=ALU.mult)
                npre = sp.tile([128, 32], BF, tag="snp")
                nc.vector.tensor_tensor(
                    npre.rearrange("p (g r) -> p g r", g=4),
                    t1.rearrange("p (g r) -> p g r", g=4),
                    psgT3[:, 8:12, t * 8:(t + 1) * 8], op=ALU.add)
                omz = sp.tile([128, 32], BF, tag="som")
                nc.vector.tensor_scalar(out=omz, in0=rz[:, 32:64], scalar1=-1.0,
                                        scalar2=1.0, op0=ALU.mult, op1=ALU.add)
                zh = sp.tile([128, 32], BF, tag="szh")
                nc.gpsimd.tensor_tensor(zh, rz[:, 32:64], hprev, op=ALU.mult)
                nnT = sp.tile([128, 32], BF, tag="snn")
                nc.scalar.activation(nnT, npre, AF.Tanh)
                nom = sp.tile([128, 32], BF, tag="snm")
                nc.vector.tensor_tensor(nom, nnT, omz, op=ALU.mult)
                nc.vector.tensor_add(h_new, nom, zh)

                # batch layout for the attention post-pass
                nc.scalar.copy(
                    hsbT.rearrange("p (c r) -> p c r", c=4)[:, :, 8 * t:8 * (t + 1)],
                    h_new.rearrange("p (c r) -> p c r", c=4))

                # keep-warm dummy (depends on h_new so it lands in the stall)
                if t >= 1:
                    nc.tensor.matmul(sdum[0:32, 0:256], lhsT=h_new[:, 0:32],
                                     rhs=waT[:, 0:256], start=True, stop=True,
                                     skip_group_check=True)

            # rebuild batch-major hsb from hsbT (4 transposes)
            pth = pgsp.tile([128, 512], BF, tag="pth")
            for j in range(4):
                nc.tensor.transpose(pth[:, j * 128:(j + 1) * 128],
                                    in_=hsbT[:, j * 128:(j + 1) * 128],
                                    identity=ident)
            nc.vector.tensor_copy(hsb[:, 0:256], pth[:, 0:256])
            nc.scalar.copy(hsb[:, 256:512], pth[:, 256:512])

          # ---- sentence attention post-pass (batch rows = (s,d)) ----
          with tc.tile_pool(name="pps", bufs=1, space="PSUM") as ppsp:
            pu2 = ppsp.tile([128, 512], F32, tag="pu2")
            nc.tensor.matmul(pu2, lhsT=ones, rhs=sbarow, start=True, stop=False)
            for j in range(4):
                nc.tensor.matmul(pu2, lhsT=hsbT[:, j * 128:(j + 1) * 128],
                                 rhs=sawT[:, j * 512:(j + 1) * 512],
                                 start=False, stop=(j == 3))
            u2 = sp.tile([128, 512], BF, tag="u2")
            nc.scalar.activation(u2, pu2, AF.Tanh)
            s_sc = sp.tile([128, 1], F32, tag="s_sc")
            scr2 = sp.tile([128, 512], BF, tag="scr2")
            nc.vector.scalar_tensor_tensor(
                out=scr2, in0=u2, scalar=1.0, in1=svb,
                op0=ALU.mult, op1=ALU.mult, accum_out=s_sc)
            ew2 = sp.tile([128, 1], F32, tag="ew2")
            nc.scalar.activation(ew2, s_sc, AF.Exp)
            # per-doc sums via indicator matmul, then broadcast back
            pds = ppsp.tile([8, 1], F32, tag="pds")
            nc.tensor.matmul(pds, lhsT=ind8f, rhs=ew2, start=True, stop=True)
            dsum = sp.tile([8, 1], F32, tag="dsum")
            nc.vector.tensor_copy(dsum, pds)
            rds = sp.tile([8, 1], F32, tag="rds")
            nc.vector.reciprocal(rds, dsum)
            prb = ppsp.tile([128, 1], F32, tag="prb")
            nc.tensor.matmul(prb, lhsT=ind8T, rhs=rds, start=True, stop=True)
            aw = sp.tile([128, 1], F32, tag="aw")
            nc.vector.tensor_tensor(aw, ew2, prb, op=ALU.mult)
            awd = sp.tile([128, 8], BF, tag="awd")
            nc.vector.tensor_scalar_mul(awd, ind8, aw)
            pdoc = ppsp.tile([8, 512], F32, tag="pdoc")
            nc.tensor.matmul(pdoc, lhsT=awd, rhs=hsb, start=True, stop=True)
            doc = sp.tile([8, 512], BF, tag="doc")
            nc.scalar.copy(doc, pdoc)

            # ---- classifier + log_softmax ----
            ptd = ppsp.tile([128, 32], BF, tag="ptd")
            for j in range(4):
                nc.tensor.transpose(ptd[:, j * 8:(j + 1) * 8],
                                    in_=doc[:, j * 128:(j + 1) * 128],
                                    identity=ident[0:8, 0:8])
            docT = sp.tile([128, 32], BF, tag="docT")
            nc.vector.tensor_copy(docT, ptd)
            pl = ppsp.tile([8, NCLS], F32, tag="pl")
            nc.tensor.matmul(pl, lhsT=ones[:, 0:8], rhs=fcb,
                             start=True, stop=False)
            for j in range(4):
                nc.tensor.matmul(pl, lhsT=docT[:, j * 8:(j + 1) * 8],
                                 rhs=fcwT[:, j * NCLS:(j + 1) * NCLS],
                                 start=False, stop=(j == 3))
            nmx2 = sp.tile([8, 1], F32, tag="nmx2")
            nc.vector.tensor_reduce(nmx2, pl, axis=mybir.AxisListType.X,
                                    op=ALU.max, negate=True)
            e2 = sp.tile([8, NCLS], F32, tag="e2")
            se2 = sp.tile([8, 1], F32, tag="se2")
            nc.scalar.activation(e2, pl, AF.Exp, bias=nmx2, accum_out=se2)
            lse = sp.tile([8, 1], F32, tag="lse")
            nc.scalar.activation(lse, se2, AF.Ln)
            out_sb = sp.tile([8, NCLS], F32, tag="out_sb")
            nc.vector.tensor_scalar(out=out_sb, in0=pl, scalar1=nmx2,
                                    scalar2=lse, op0=ALU.add, op1=ALU.subtract)
            nc.sync.dma_start(out=dram("out"), in_=out_sb)


# ---------------------------------------------------------------------------
# host side
# ---------------------------------------------------------------------------

def _prep_inputs(inputs):
    """Build the per-core in_maps (host preprocessing + sharding)."""
    f32 = np.float32
    emb = np.asarray(inputs["emb"], f32)
    w_Wih = np.asarray(inputs["w_Wih"], f32)
    w_Whh = np.asarray(inputs["w_Whh"], f32)
    w_bih = np.asarray(inputs["w_bih"], f32)
    w_bhh = np.asarray(inputs["w_bhh"], f32)
    wa_W = np.asarray(inputs["wa_W"], f32)
    wa_b = np.asarray(inputs["wa_b"], f32)
    wa_v = np.asarray(inputs["wa_v"], f32)
    s_Wih = np.asarray(inputs["s_Wih"], f32)
    s_Whh = np.asarray(inputs["s_Whh"], f32)
    s_bih = np.asarray(inputs["s_bih"], f32)
    s_bhh = np.asarray(inputs["s_bhh"], f32)
    sa_W = np.asarray(inputs["sa_W"], f32)
    sa_b = np.asarray(inputs["sa_b"], f32)
    sa_v = np.asarray(inputs["sa_v"], f32)
    fc_W = np.asarray(inputs["fc_W"], f32)
    fc_b = np.asarray(inputs["fc_b"], f32)
    tokens = np.asarray(inputs["tokens"])

    def b(x):
        return np.ascontiguousarray(x.astype(bf16))

    # folded gather table G [V, 1536] = [rz0 | rz1 | n0 | n1]
    g0 = emb @ w_Wih[0].T + w_bih[0]
    g0[:, :512] += w_bhh[0][:512]
    g1 = emb @ w_Wih[1].T + w_bih[1]
    g1[:, :512] += w_bhh[1][:512]
    G = np.concatenate([g0[:, :512], g1[:, :512], g0[:, 512:], g1[:, 512:]], 1)

    whhT = np.stack([w_Whh[0].T[:128], w_Whh[0].T[128:],
                     w_Whh[1].T[:128], w_Whh[1].T[128:]])  # [4,128,768]
    brow = np.concatenate([w_bhh[0][512:], w_bhh[1][512:]])[None, :]
    vbc = np.broadcast_to(wa_v, (128, 512))

    # sentence input-proj table [512, 1536] with same col layout; bias row
    sg0 = s_Wih[0].T  # [512, 768]
    sg1 = s_Wih[1].T
    swihT = np.concatenate([sg0[:, :512], sg1[:, :512],
                            sg0[:, 512:], sg1[:, 512:]], 1)
    sprow = np.concatenate([
        s_bih[0][:512] + s_bhh[0][:512],
        s_bih[1][:512] + s_bhh[1][:512],
        s_bih[0][512:], s_bih[1][512:]])[None, :]
    swhhT = np.stack([s_Whh[0].T[:128], s_Whh[0].T[128:],
                      s_Whh[1].T[:128], s_Whh[1].T[128:]])
    # feature-major weight chunks for the sentence GRU, slot order
    # [d0r0 d0r1 d1r0 d1r1 d0z0 d0z1 d1z0 d1z1 d0n0 d0n1 d1n0 d1n1] x K
    goff = {"r": 0, "z": 256, "n": 512}
    fch = []
    for g in ("r", "z", "n"):
        for d in range(2):
            for m in range(2):
                for k in range(2):
                    fch.append(s_Whh[d][goff[g] + m * 128:goff[g] + (m + 1) * 128,
                                        k * 128:(k + 1) * 128].T)
    swhhF = np.stack(fch)  # [24, 128, 128]
    sbrow = np.concatenate([s_bhh[0][512:], s_bhh[1][512:]])[None, :]
    svbc = np.broadcast_to(sa_v, (128, 512))

    ind = np.zeros((128, 8), f32)
    for row in range(128):
        ind[row, row % 8] = 1.0

    shared = {
        "G": b(G), "whhT": b(whhT), "brow": b(brow),
        "waT": b(wa_W.T), "barow": b(wa_b[None, :]), "vb": b(vbc),
        "swihT": b(swihT), "sprow": b(sprow), "swhhT": b(swhhT),
        "swhhF": b(swhhF),
        "sbrowF": b(sbrow.reshape(4, 128)),
        "bones": b(np.repeat(np.eye(4, dtype=f32), 8, axis=1)),
        "sbrow": b(sbrow), "sawT": b(sa_W.T), "sbarow": b(sa_b[None, :]),
        "svb": b(svbc), "fcwT": b(fc_W.T), "fcb": b(fc_b[None, :]),
        "ind8": b(ind), "ind8f": np.ascontiguousarray(ind),
        "ind8T": np.ascontiguousarray(ind.T),
    }
    in_maps = []
    for c in range(NCORES):
        # word-row p = s*8 + doc  (so sentence step s owns partition rows
        # [s*8:(s+1)*8] of the batch-major sentence matrix)
        tk = np.ascontiguousarray(
            np.transpose(tokens[c * BC:(c + 1) * BC], (1, 0, 2))
            .reshape(NW, W).astype(np.int32))
        in_maps.append({**shared, "toks": tk})
    return in_maps


_NC_CACHE = {}


def _get_nc():
    if "nc" not in _NC_CACHE:
        _NC_CACHE["nc"] = _build_program()
    return _NC_CACHE["nc"]


def kernel(**inputs) -> np.ndarray:
    nc = _get_nc()
    in_maps = _prep_inputs(inputs)
    res = bass_utils.run_bass_kernel_spmd(nc, in_maps, core_ids=list(range(NCORES)))
    outs = []
    for c in range(NCORES):
        o = np.asarray(res.results[c]["out"], np.float32)
        outs.append(o)
    return np.concatenate(outs, 0)


# revision 44
# speedup vs baseline: 1.1745x; 1.1745x over previous
"""HAN (hierarchical attention network) forward pass on 8 TRN2 NeuronCores.

Strategy
--------
Data-parallel over batch: each core handles 8 documents = 128 sentences =
4096 tokens, fully independently (no collectives). Inside a core:

* Embedding lookup + word-GRU input projection folded on host:
  gi = (emb @ Wih.T)[tokens], gathered per step with indirect DMA.
* Word bi-GRU, batch-major [128 sentences, feat]. Per-direction gate math
  (FD=256 elementwise ops) so the two independent direction recurrences
  pipeline across ACT/DVE. PE program order is arranged so that the
  next step's gi injection and the previous step's attention matmuls
  execute while the gate-math chain runs (keeps PE warm, no HAM
  re-throttle).
* Word attention: scores accumulated in-loop (u = tanh(h Wa + ba) lagged
  one step; u.v via stt-accum on GpSimd). Softmax without max-subtraction
  (host-validated score bounds) as an incremental exp-weighted running
  sum, so the end-of-loop softmax valley disappears.
* Sentence bi-GRU: same structure, batch 8. Sentence attention is a
  post-pass batch GEMM over a [128 rows=(step,doc)] layout; the
  per-document softmax is done with indicator matmuls (segmented sum +
  broadcast-back), and the weighted sum is a single matmul with a
  masked-diagonal lhsT.

Compute dtype bf16 (fp32 PSUM accumulation + fp32 attention
accumulators).
"""

import numpy as np
import ml_dtypes

import concourse.bass as bass
import concourse.mybir as mybir
import concourse.tile as tile
from concourse import bacc, bass_utils
from concourse.masks import make_identity

BF = mybir.dt.bfloat16
F32 = mybir.dt.float32
AF = mybir.ActivationFunctionType
ALU = mybir.AluOpType
bf16 = ml_dtypes.bfloat16

V, E = 50000, 300
HW_, HS_ = 256, 256
NCLS = 10
B, S, W = 64, 16, 32
NCORES = 8
BC = B // NCORES          # docs per core = 8
NW = BC * S               # word-level batch per core = 128
GW = 3 * HW_              # 768


def _build_program():
    nc = bacc.Bacc(
        "TRN2",
        target_bir_lowering=False,
        debug=False,
        enable_asserts=False,
        num_devices=NCORES,
    )

    # ---- DRAM I/O ----
    h = {}
    h["G"] = nc.dram_tensor("G", [V, 1536], BF, kind="ExternalInput")
    h["toks"] = nc.dram_tensor("toks", [128, 32], mybir.dt.int32, kind="ExternalInput")
    h["whhT"] = nc.dram_tensor("whhT", [4, 128, GW], BF, kind="ExternalInput")
    h["brow"] = nc.dram_tensor("brow", [1, 512], BF, kind="ExternalInput")
    h["waT"] = nc.dram_tensor("waT", [512, 512], BF, kind="ExternalInput")
    h["barow"] = nc.dram_tensor("barow", [1, 512], BF, kind="ExternalInput")
    h["vb"] = nc.dram_tensor("vb", [128, 512], BF, kind="ExternalInput")
    h["swihT"] = nc.dram_tensor("swihT", [512, 1536], BF, kind="ExternalInput")
    h["sprow"] = nc.dram_tensor("sprow", [1, 1536], BF, kind="ExternalInput")
    h["swhhT"] = nc.dram_tensor("swhhT", [4, 128, GW], BF, kind="ExternalInput")
    h["swhhF"] = nc.dram_tensor("swhhF", [24, 128, 128], BF, kind="ExternalInput")
    h["sbrowF"] = nc.dram_tensor("sbrowF", [4, 128], BF, kind="ExternalInput")
    h["bones"] = nc.dram_tensor("bones", [4, 32], BF, kind="ExternalInput")
    h["sbrow"] = nc.dram_tensor("sbrow", [1, 512], BF, kind="ExternalInput")
    h["sawT"] = nc.dram_tensor("sawT", [512, 512], BF, kind="ExternalInput")
    h["sbarow"] = nc.dram_tensor("sbarow", [1, 512], BF, kind="ExternalInput")
    h["svb"] = nc.dram_tensor("svb", [128, 512], BF, kind="ExternalInput")
    h["ind8"] = nc.dram_tensor("ind8", [128, 8], BF, kind="ExternalInput")
    h["ind8f"] = nc.dram_tensor("ind8f", [128, 8], F32, kind="ExternalInput")
    h["ind8T"] = nc.dram_tensor("ind8T", [8, 128], F32, kind="ExternalInput")
    h["fcwT"] = nc.dram_tensor("fcwT", [512, NCLS], BF, kind="ExternalInput")
    h["fcb"] = nc.dram_tensor("fcb", [1, NCLS], BF, kind="ExternalInput")
    h["out"] = nc.dram_tensor("out", [BC, NCLS], F32, kind="ExternalOutput")

    with tile.TileContext(nc) as tc:
        _body(nc, tc, h)
    nc.compile()
    return nc


def _body(nc, tc, handles):
    def dram(name):
        return handles[name].ap()

    G_ap = dram("G")
    with tc.tile_pool(name="const", bufs=1) as cp:
        # ---- constants / weights in SBUF ----
        ident = cp.tile([128, 128], BF)
        make_identity(nc, ident)
        ones = cp.tile([1, 128], BF)
        nc.gpsimd.memset(ones, 1.0)

        toks = cp.tile([128, 32], mybir.dt.int32)
        nc.sync.dma_start(out=toks, in_=dram("toks"))
        whh = cp.tile([128, 4 * GW], BF)  # (d0k0 d0k1 d1k0 d1k1); [rz(512) n(256)]
        for j in range(4):
            nc.sync.dma_start(out=whh[:, j * GW:(j + 1) * GW],
                              in_=dram("whhT")[j])
        brow = cp.tile([1, 512], BF)
        nc.sync.dma_start(out=brow, in_=dram("brow"))
        waT = cp.tile([128, 4 * 512], BF)
        for j in range(4):
            nc.sync.dma_start(out=waT[:, j * 512:(j + 1) * 512],
                              in_=dram("waT")[j * 128:(j + 1) * 128, :])
        barow = cp.tile([1, 512], BF)
        nc.sync.dma_start(out=barow, in_=dram("barow"))
        vb = cp.tile([128, 512], BF)
        nc.sync.dma_start(out=vb, in_=dram("vb"))

        swihT = cp.tile([128, 4 * 1536], BF)
        for j in range(4):
            nc.sync.dma_start(out=swihT[:, j * 1536:(j + 1) * 1536],
                              in_=dram("swihT")[j * 128:(j + 1) * 128, :])
        sprow = cp.tile([1, 1536], BF)
        nc.sync.dma_start(out=sprow, in_=dram("sprow"))
        swhhF = cp.tile([128, 24 * 128], BF)
        for j in range(24):
            nc.sync.dma_start(out=swhhF[:, j * 128:(j + 1) * 128],
                              in_=dram("swhhF")[j])
        sbrow = cp.tile([1, 512], BF)
        nc.sync.dma_start(out=sbrow, in_=dram("sbrow"))
        sbrowF = cp.tile([4, 128], BF)
        nc.sync.dma_start(out=sbrowF, in_=dram("sbrowF"))
        bones = cp.tile([4, 32], BF)
        nc.sync.dma_start(out=bones, in_=dram("bones"))
        sawT = cp.tile([128, 4 * 512], BF)
        for j in range(4):
            nc.sync.dma_start(out=sawT[:, j * 512:(j + 1) * 512],
                              in_=dram("sawT")[j * 128:(j + 1) * 128, :])
        sbarow = cp.tile([1, 512], BF)
        nc.sync.dma_start(out=sbarow, in_=dram("sbarow"))
        svb = cp.tile([128, 512], BF)
        nc.sync.dma_start(out=svb, in_=dram("svb"))
        ind8 = cp.tile([128, 8], BF)
        nc.sync.dma_start(out=ind8, in_=dram("ind8"))
        ind8f = cp.tile([128, 8], F32)
        nc.sync.dma_start(out=ind8f, in_=dram("ind8f"))
        ind8T = cp.tile([8, 128], F32)
        nc.sync.dma_start(out=ind8T, in_=dram("ind8T"))
        fcwT = cp.tile([128, 4 * NCLS], BF)
        for j in range(4):
            nc.sync.dma_start(out=fcwT[:, j * NCLS:(j + 1) * NCLS],
                              in_=dram("fcwT")[j * 128:(j + 1) * 128, :])
        fcb = cp.tile([1, NCLS], BF)
        nc.sync.dma_start(out=fcb, in_=dram("fcb"))

        # ---- persistent state ----
        hw_hist = cp.tile([128, 33 * 512], BF)   # h_t history, slot 0 = zeros
        nc.gpsimd.memset(hw_hist[:, 0:512], 0.0)
        hT0 = cp.tile([128, 512], BF)            # transposed h state, step -1
        nc.gpsimd.memset(hT0, 0.0)
        scores = cp.tile([128, 32], F32)
        ew = cp.tile([128, 32], F32)             # exp(scores)
        separts = cp.tile([128, 8], F32)         # partial exp sums (per 4-batch)
        wacc = cp.tile([128, 512], F32)          # running exp-weighted h sum
        nc.gpsimd.memset(wacc, 0.0)
        sent = cp.tile([128, 512], BF)           # word-attention output
        sgi = cp.tile([128, 1536], BF)           # sentence-GRU input projections
        psgT = cp.tile([128, 12 * 128], BF)      # transposed gi: gate-chunk x rows
        hsf_hist = cp.tile([128, 17 * 32], BF)   # feature-major h^T history
        nc.gpsimd.memset(hsf_hist[:, 0:32], 0.0)
        hsb = cp.tile([128, 512], BF)            # sentence h, batch rows (s,d)
        hsbT = cp.tile([128, 4 * 128], BF)       # transposed: feat-chunk x rows

        # ================= word stage =================
        with tc.tile_pool(name="wp", bufs=3) as wp, \
             tc.tile_pool(name="wgi", bufs=5) as wgi, \
             tc.tile_pool(name="pg", bufs=2, space="PSUM") as pgp, \
             tc.tile_pool(name="pn2", bufs=1, space="PSUM") as pnp, \
             tc.tile_pool(name="pwa", bufs=1, space="PSUM") as pwp, \
             tc.tile_pool(name="pt", bufs=1, space="PSUM") as ptp, \
             tc.tile_pool(name="pu", bufs=1, space="PSUM") as pup:

            PRE = 3  # gather prefetch depth
            gi_tiles = {}
            for t in range(PRE):
                g = wgi.tile([128, 1536], BF, tag="gi")
                nc.gpsimd.indirect_dma_start(
                    out=g[:, :], out_offset=None, in_=G_ap[:, :],
                    in_offset=bass.IndirectOffsetOnAxis(ap=toks[:, t:t + 1], axis=0),
                )
                gi_tiles[t] = g

            def inject(t):
                """Open PSUM accumulation groups for step t with gi + biases."""
                gi = gi_tiles[t]
                a = pgp.tile([128, 512], F32, tag="pgd0")
                b = pgp.tile([128, 512], F32, tag="pgd1")
                p = pnp.tile([128, 512], F32, tag="pn")
                nc.tensor.matmul(a, lhsT=ident, rhs=gi[:, 0:512],
                                 start=True, stop=False)
                nc.tensor.matmul(b, lhsT=ident, rhs=gi[:, 512:1024],
                                 start=True, stop=False)
                nc.tensor.matmul(p, lhsT=ones, rhs=brow,
                                 start=True, stop=False)
                return a, b, p

            pg_next = inject(0)

            prev_hT = hT0
            prev_u = None
            wsum_p = 0  # next pending weighted-sum step

            for t in range(32):
                pga, pgb, pn = pg_next
                gi = gi_tiles.pop(t)

                # --- PE: recurrent matmuls for step t (need prev_hT) ---
                for dd in range(2):
                    tgt = pga if dd == 0 else pgb
                    for k in range(2):
                        lhs = prev_hT[:, (dd * 2 + k) * 128:(dd * 2 + k + 1) * 128]
                        w = whh[:, (dd * 2 + k) * GW:(dd * 2 + k + 1) * GW]
                        nc.tensor.matmul(tgt, lhsT=lhs, rhs=w[:, 0:512],
                                         start=False, stop=(k == 1))
                for dd in range(2):
                    for k in range(2):
                        lhs = prev_hT[:, (dd * 2 + k) * 128:(dd * 2 + k + 1) * 128]
                        w = whh[:, (dd * 2 + k) * GW:(dd * 2 + k + 1) * GW]
                        nc.tensor.matmul(pn[:, dd * 256:(dd + 1) * 256],
                                         lhsT=lhs, rhs=w[:, 512:768],
                                         start=False,
                                         stop=(dd == 1 and k == 1))

                # --- PE: attention matmuls for step t-1 (fills stall) ---
                if t >= 1:
                    pu = pup.tile([128, 512], F32, tag="pu")
                    nc.tensor.matmul(pu, lhsT=ones, rhs=barow,
                                     start=True, stop=False)
                    for j in range(4):
                        nc.tensor.matmul(pu, lhsT=prev_hT[:, j * 128:(j + 1) * 128],
                                         rhs=waT[:, j * 512:(j + 1) * 512],
                                         start=False, stop=(j == 3))

                # --- PE: inject step t+1 (gi ready from DMA) ---
                if t < 31:
                    pg_next = inject(t + 1)

                # --- DMA: prefetch gather for step t+PRE ---
                if t + PRE < 32:
                    g = wgi.tile([128, 1536], BF, tag="gi")
                    nc.gpsimd.indirect_dma_start(
                        out=g[:, :], out_offset=None, in_=G_ap[:, :],
                        in_offset=bass.IndirectOffsetOnAxis(
                            ap=toks[:, t + PRE:t + PRE + 1], axis=0),
                    )
                    gi_tiles[t + PRE] = g

                # --- gate math, per direction (ACT/DVE pipeline) ---
                # h' = (1-z).n + z.h_prev with (1-z) on DVE and z.h_prev on
                # GpSimd, both off the critical chain.
                h_prev = hw_hist[:, t * 512:(t + 1) * 512]
                h_new = hw_hist[:, (t + 1) * 512:(t + 2) * 512]
                rz = wp.tile([128, 1024], BF, tag="rz")  # [r0 z0 r1 z1]
                nn = wp.tile([128, 512], BF, tag="nn")   # [n0 n1]
                for dd in range(2):
                    pgd = pga if dd == 0 else pgb
                    rzd = rz[:, dd * 512:(dd + 1) * 512]
                    nc.scalar.activation(rzd, pgd, AF.Sigmoid)
                    r_d = rz[:, dd * 512:dd * 512 + 256]
                    z_d = rz[:, dd * 512 + 256:(dd + 1) * 512]
                    t1 = wp.tile([128, 256], BF, tag=f"t1{dd}")
                    nc.vector.tensor_tensor(t1, r_d, pn[:, dd * 256:(dd + 1) * 256],
                                            op=ALU.mult)
                    npre = wp.tile([128, 256], BF, tag=f"np{dd}")
                    nc.vector.tensor_add(npre, t1,
                                         gi[:, 1024 + dd * 256:1024 + (dd + 1) * 256])
                    omz = wp.tile([128, 256], BF, tag=f"om{dd}")
                    nc.vector.tensor_scalar(out=omz, in0=z_d, scalar1=-1.0,
                                            scalar2=1.0, op0=ALU.mult, op1=ALU.add)
                    zh = wp.tile([128, 256], BF, tag=f"zh{dd}")
                    nc.gpsimd.tensor_tensor(
                        zh, z_d, h_prev[:, dd * 256:(dd + 1) * 256], op=ALU.mult)
                    nnd = nn[:, dd * 256:(dd + 1) * 256]
                    nc.scalar.activation(nnd, npre, AF.Tanh)
                    nom = wp.tile([128, 256], BF, tag=f"nm{dd}")
                    nc.vector.tensor_tensor(nom, nnd, omz, op=ALU.mult)
                    nc.vector.tensor_add(h_new[:, dd * 256:(dd + 1) * 256], nom, zh)

                # --- u(t-1) = tanh(pu); score via stt-accum on GpSimd ---
                if t >= 1:
                    u = wp.tile([128, 512], BF, tag="u")
                    nc.scalar.activation(u, pu, AF.Tanh)
                    scr = wp.tile([128, 512], BF, tag="scr")
                    nc.vector.scalar_tensor_tensor(
                        out=scr, in0=u, scalar=1.0, in1=vb,
                        op0=ALU.mult, op1=ALU.mult,
                        accum_out=scores[:, t - 1:t])
                    prev_u = u

                # --- batched exp of scores, every 4 completed steps ---
                # exp(s) = 1/sigmoid(-s) - 1 (stays in the sigmoid table set;
                # a real Exp would force two ACT_TABLE_LOADs per batch)
                done = t  # scores available: 0..t-1 (scr for t-1 just queued)
                if done % 4 == 0 and done > 0:
                    j = done // 4 - 1
                    sl = slice(j * 4, (j + 1) * 4)
                    nc.scalar.activation(separts[:, 0:4], scores[:, sl],
                                         AF.Sigmoid, scale=-1.0)
                    nc.vector.reciprocal(separts[:, 4:8], separts[:, 0:4])
                    nc.vector.tensor_scalar_add(ew[:, sl], separts[:, 4:8], -1.0)
                # --- lagged weighted-sum: diag matmul into PSUM chunk ---
                # (runs on PE during the gate-math stall; replaces DVE stt)
                if wsum_p < (t // 4) * 4:
                    s = wsum_p
                    dg = wp.tile([128, 128], BF, tag="dg")
                    nc.vector.tensor_scalar_mul(dg, ident, ew[:, s:s + 1])
                    if s % 4 == 0:
                        pwa = pwp.tile([128, 512], F32, tag="pwa")
                    nc.tensor.matmul(pwa, lhsT=dg,
                                     rhs=hw_hist[:, (s + 1) * 512:(s + 2) * 512],
                                     start=(s % 4 == 0), stop=(s % 4 == 3))
                    if s % 4 == 3:
                        nc.vector.tensor_tensor(wacc, wacc, pwa, op=ALU.add)
                    wsum_p += 1

                # --- keep-warm dummy matmuls (execute during the stall) ---
                if t >= 1:
                    nc.tensor.matmul(pu[:, 0:512], lhsT=ident,
                                     rhs=waT[:, 512:1024], start=True, stop=True,
                                     skip_group_check=True)
                    nc.tensor.matmul(pu[:, 0:512], lhsT=ident,
                                     rhs=waT[:, 1024:1536], start=True, stop=True,
                                     skip_group_check=True)

                # --- transpose h_new -> hT (stall point, last in PE queue) ---
                pt = ptp.tile([128, 512], BF, tag="pt")
                for j in range(4):
                    nc.tensor.transpose(pt[:, j * 128:(j + 1) * 128],
                                        in_=h_new[:, j * 128:(j + 1) * 128],
                                        identity=ident)
                hT = wp.tile([128, 512], BF, tag="hT")
                nc.vector.tensor_copy(hT[:, 0:256], pt[:, 0:256])
                nc.scalar.copy(hT[:, 256:512], pt[:, 256:512])
                prev_hT = hT

            # ---- word epilogue: attention for t=31 + remaining wsum ----
            pu = pup.tile([128, 512], F32, tag="pu")
            nc.tensor.matmul(pu, lhsT=ones, rhs=barow, start=True, stop=False)
            for j in range(4):
                nc.tensor.matmul(pu, lhsT=prev_hT[:, j * 128:(j + 1) * 128],
                                 rhs=waT[:, j * 512:(j + 1) * 512],
                                 start=False, stop=(j == 3))
            u = wp.tile([128, 512], BF, tag="u")
            nc.scalar.activation(u, pu, AF.Tanh)
            scr = wp.tile([128, 512], BF, tag="scr")
            nc.vector.scalar_tensor_tensor(
                out=scr, in0=u, scalar=1.0, in1=vb,
                op0=ALU.mult, op1=ALU.mult, accum_out=scores[:, 31:32])
            nc.scalar.activation(separts[:, 0:4], scores[:, 28:32],
                                 AF.Sigmoid, scale=-1.0)
            nc.vector.reciprocal(separts[:, 4:8], separts[:, 0:4])
            nc.vector.tensor_scalar_add(ew[:, 28:32], separts[:, 4:8], -1.0)
            # catch up weighted sum (diag matmuls)
            while wsum_p < 32:
                s = wsum_p
                dg = wp.tile([128, 128], BF, tag="dg")
                nc.vector.tensor_scalar_mul(dg, ident, ew[:, s:s + 1])
                if s % 4 == 0:
                    pwa = pwp.tile([128, 512], F32, tag="pwa")
                nc.tensor.matmul(pwa, lhsT=dg,
                                 rhs=hw_hist[:, (s + 1) * 512:(s + 2) * 512],
                                 start=(s % 4 == 0), stop=(s % 4 == 3))
                if s % 4 == 3:
                    nc.vector.tensor_tensor(wacc, wacc, pwa, op=ALU.add)
                wsum_p += 1
            # normalize: sent = wacc / sum(exp)
            se = wp.tile([128, 1], F32, tag="se")
            nc.vector.tensor_reduce(se, ew, axis=mybir.AxisListType.X,
                                    op=ALU.add)
            rse = wp.tile([128, 1], F32, tag="rse")
            nc.vector.reciprocal(rse, se)
            nc.vector.tensor_scalar_mul(sent, wacc, rse)

        # ---- sent -> sentT + sentence input projections ----
        with tc.tile_pool(name="mid", bufs=1) as mp, \
             tc.tile_pool(name="pmid", bufs=1, space="PSUM") as pmp:
            ptm = pmp.tile([128, 512], BF, tag="ptm")
            for j in range(4):
                nc.tensor.transpose(ptm[:, j * 128:(j + 1) * 128],
                                    in_=sent[:, j * 128:(j + 1) * 128],
                                    identity=ident)
            sentT = mp.tile([128, 512], BF)
            nc.vector.tensor_copy(sentT[:, 0:256], ptm[:, 0:256])
            nc.scalar.copy(sentT[:, 256:512], ptm[:, 256:512])

            psg = pmp.tile([128, 1536], F32, tag="psg")
            for ns in range(3):
                sl = slice(ns * 512, (ns + 1) * 512)
                nc.tensor.matmul(psg[:, sl], lhsT=ones, rhs=sprow[:, sl],
                                 start=True, stop=False)
                for k in range(4):
                    nc.tensor.matmul(psg[:, sl],
                                     lhsT=sentT[:, k * 128:(k + 1) * 128],
                                     rhs=swihT[:, k * 1536 + ns * 512:
                                               k * 1536 + (ns + 1) * 512],
                                     start=False, stop=(k == 3))
            nc.scalar.copy(sgi[:, 0:512], psg[:, 0:512])
            nc.scalar.copy(sgi[:, 512:1024], psg[:, 512:1024])
            nc.vector.tensor_copy(sgi[:, 1024:1536], psg[:, 1024:1536])
            # transpose sgi into feature-major gate-chunk slots:
            # [d0r0 d0r1 d1r0 d1r1 d0z0 d0z1 d1z0 d1z1 d0n0 d0n1 d1n0 d1n1]
            slot_src = [0, 128, 512, 640, 256, 384, 768, 896,
                        1024, 1152, 1280, 1408]
            ptg = pmp.tile([128, 1536], BF, tag="ptg")
            for j, src in enumerate(slot_src):
                nc.tensor.transpose(ptg[:, j * 128:(j + 1) * 128],
                                    in_=sgi[:, src:src + 128],
                                    identity=ident)
            nc.scalar.copy(psgT[:, 0:512], ptg[:, 0:512])
            nc.vector.tensor_copy(psgT[:, 512:1024], ptg[:, 512:1024])
            nc.scalar.copy(psgT[:, 1024:1536], ptg[:, 1024:1536])

        # ================= sentence stage =================
        # Feature-major sentence GRU: state h^T [128 feat(chunk d,m), 8 docs],
        # weight-stationary rec matmuls (N=8), gate math at FD<=64.
        with tc.tile_pool(name="sp", bufs=3) as sp:
          with tc.tile_pool(name="pgs", bufs=2, space="PSUM") as pgsp, \
               tc.tile_pool(name="psd", bufs=1, space="PSUM") as psdp:
            sdum = psdp.tile([128, 512], F32, tag="sdum")
            psgT3 = psgT.rearrange("p (g r) -> p g r", g=12)

            for t in range(16):
                hprev = hsf_hist[:, t * 32:(t + 1) * 32]
                h_new = hsf_hist[:, (t + 1) * 32:(t + 2) * 32]
                prz_full = pgsp.tile([128, 512], F32, tag="prz")
                pnn_full = pgsp.tile([128, 512], F32, tag="pnn")
                prz = prz_full[:, 0:64]
                pnn = pnn_full[:, 0:32]
                # n-gate bias opener: pnn[p, i*8+j] = bhh_n[i*128+p]
                nc.tensor.matmul(pnn, lhsT=sbrowF, rhs=bones,
                                 start=True, stop=False)
                # recurrent: rz slots 0..7, n slots 0..3 (chunk (d, m), K (d, k))
                for s in range(8):
                    d = (s % 4) // 2
                    for k in range(2):
                        w = swhhF[:, (s * 2 + k) * 128:(s * 2 + k + 1) * 128]
                        nc.tensor.matmul(prz[:, s * 8:(s + 1) * 8], lhsT=w,
                                         rhs=hprev[:, (d * 2 + k) * 8:
                                                   (d * 2 + k + 1) * 8],
                                         start=(k == 0), stop=(k == 1))
                for i in range(4):
                    d = i // 2
                    for k in range(2):
                        w = swhhF[:, (16 + i * 2 + k) * 128:
                                  (16 + i * 2 + k + 1) * 128]
                        nc.tensor.matmul(pnn[:, i * 8:(i + 1) * 8], lhsT=w,
                                         rhs=hprev[:, (d * 2 + k) * 8:
                                                   (d * 2 + k + 1) * 8],
                                         start=False,
                                         stop=(i == 3 and k == 1))

                # gate math (tiny FD)
                rzp = sp.tile([128, 64], BF, tag="srzp")
                nc.vector.tensor_tensor(
                    rzp.rearrange("p (g r) -> p g r", g=8),
                    prz.rearrange("p (g r) -> p g r", g=8),
                    psgT3[:, 0:8, t * 8:(t + 1) * 8], op=ALU.add)
                rz = sp.tile([128, 64], BF, tag="srz")
                nc.scalar.activation(rz, rzp, AF.Sigmoid)
                t1 = sp.tile([128, 32], BF, tag="st1")
                nc.vector.tensor_tensor(t1, rz[:, 0:32], pnn, op=ALU.mult)
                npre = sp.tile([128, 32], BF, tag="snp")
                nc.vector.tensor_tensor(
                    npre.rearrange("p (g r) -> p g r", g=4),
                    t1.rearrange("p (g r) -> p g r", g=4),
                    psgT3[:, 8:12, t * 8:(t + 1) * 8], op=ALU.add)
                omz = sp.tile([128, 32], BF, tag="som")
                nc.vector.tensor_scalar(out=omz, in0=rz[:, 32:64], scalar1=-1.0,
                                        scalar2=1.0, op0=ALU.mult, op1=ALU.add)
                zh = sp.tile([128, 32], BF, tag="szh")
                nc.gpsimd.tensor_tensor(zh, rz[:, 32:64], hprev, op=ALU.mult)
                nnT = sp.tile([128, 32], BF, tag="snn")
                nc.scalar.activation(nnT, npre, AF.Tanh)
                nom = sp.tile([128, 32], BF, tag="snm")
                nc.vector.tensor_tensor(nom, nnT, omz, op=ALU.mult)
                nc.vector.tensor_add(h_new, nom, zh)

                # batch layout for the attention post-pass
                nc.scalar.copy(
                    hsbT.rearrange("p (c r) -> p c r", c=4)[:, :, 8 * t:8 * (t + 1)],
                    h_new.rearrange("p (c r) -> p c r", c=4))

                # keep-warm dummy (depends on h_new so it lands in the stall)
                if t >= 1:
                    nc.tensor.matmul(sdum[0:32, 0:256], lhsT=h_new[:, 0:32],
                                     rhs=waT[:, 0:256], start=True, stop=True,
                                     skip_group_check=True)

            # rebuild batch-major hsb from hsbT (4 transposes)
            pth = pgsp.tile([128, 512], BF, tag="pth")
            for j in range(4):
                nc.tensor.transpose(pth[:, j * 128:(j + 1) * 128],
                                    in_=hsbT[:, j * 128:(j + 1) * 128],
                                    identity=ident)
            nc.vector.tensor_copy(hsb[:, 0:256], pth[:, 0:256])
            nc.scalar.copy(hsb[:, 256:512], pth[:, 256:512])

          # ---- sentence attention post-pass (batch rows = (s,d)) ----
          with tc.tile_pool(name="pps", bufs=1, space="PSUM") as ppsp:
            pu2 = ppsp.tile([128, 512], F32, tag="pu2")
            nc.tensor.matmul(pu2, lhsT=ones, rhs=sbarow, start=True, stop=False)
            for j in range(4):
                nc.tensor.matmul(pu2, lhsT=hsbT[:, j * 128:(j + 1) * 128],
                                 rhs=sawT[:, j * 512:(j + 1) * 512],
                                 start=False, stop=(j == 3))
            u2 = sp.tile([128, 512], BF, tag="u2")
            nc.scalar.activation(u2, pu2, AF.Tanh)
            s_sc = sp.tile([128, 1], F32, tag="s_sc")
            scr2 = sp.tile([128, 512], BF, tag="scr2")
            nc.vector.scalar_tensor_tensor(
                out=scr2, in0=u2, scalar=1.0, in1=svb,
                op0=ALU.mult, op1=ALU.mult, accum_out=s_sc)
            ew2 = sp.tile([128, 1], F32, tag="ew2")
            nc.scalar.activation(ew2, s_sc, AF.Exp)
            # per-doc sums via indicator matmul, then broadcast back
            pds = ppsp.tile([8, 1], F32, tag="pds")
            nc.tensor.matmul(pds, lhsT=ind8f, rhs=ew2, start=True, stop=True)
            dsum = sp.tile([8, 1], F32, tag="dsum")
            nc.vector.tensor_copy(dsum, pds)
            rds = sp.tile([8, 1], F32, tag="rds")
            nc.vector.reciprocal(rds, dsum)
            prb = ppsp.tile([128, 1], F32, tag="prb")
            nc.tensor.matmul(prb, lhsT=ind8T, rhs=rds, start=True, stop=True)
            aw = sp.tile([128, 1], F32, tag="aw")
            nc.vector.tensor_tensor(aw, ew2, prb, op=ALU.mult)
            awd = sp.tile([128, 8], BF, tag="awd")
            nc.vector.tensor_scalar_mul(awd, ind8, aw)
            pdoc = ppsp.tile([8, 512], F32, tag="pdoc")
            nc.tensor.matmul(pdoc, lhsT=awd, rhs=hsb, start=True, stop=True)
            doc = sp.tile([8, 512], BF, tag="doc")
            nc.scalar.copy(doc, pdoc)

            # ---- classifier + log_softmax ----
            ptd = ppsp.tile([128, 32], BF, tag="ptd")
            for j in range(4):
                nc.tensor.transpose(ptd[:, j * 8:(j + 1) * 8],
                                    in_=doc[:, j * 128:(j + 1) * 128],
                                    identity=ident[0:8, 0:8])
            docT = sp.tile([128, 32], BF, tag="docT")
            nc.vector.tensor_copy(docT, ptd)
            pl = ppsp.tile([8, NCLS], F32, tag="pl")
            nc.tensor.matmul(pl, lhsT=ones[:, 0:8], rhs=fcb,
                             start=True, stop=False)
            for j in range(4):
                nc.tensor.matmul(pl, lhsT=docT[:, j * 8:(j + 1) * 8],
                                 rhs=fcwT[:, j * NCLS:(j + 1) * NCLS],
                                 start=False, stop=(j == 3))
            nmx2 = sp.tile([8, 1], F32, tag="nmx2")
            nc.vector.tensor_reduce(nmx2, pl, axis=mybir.AxisListType.X,
                                    op=ALU.max, negate=True)
            e2 = sp.tile([8, NCLS], F32, tag="e2")
            se2 = sp.tile([8, 1], F32, tag="se2")
            nc.scalar.activation(e2, pl, AF.Exp, bias=nmx2, accum_out=se2)
            lse = sp.tile([8, 1], F32, tag="lse")
            nc.scalar.activation(lse, se2, AF.Ln)
            out_sb = sp.tile([8, NCLS], F32, tag="out_sb")
            nc.vector.tensor_scalar(out=out_sb, in0=pl, scalar1=nmx2,
                                    scalar2=lse, op0=ALU.add, op1=ALU.subtract)
            nc.sync.dma_start(out=dram("out"), in_=out_sb)


# ---------------------------------------------------------------------------
# host side
# ---------------------------------------------------------------------------

def _prep_inputs(inputs):
    """Build the per-core in_maps (host preprocessing + sharding)."""
    f32 = np.float32
    emb = np.asarray(inputs["emb"], f32)
    w_Wih = np.asarray(inputs["w_Wih"], f32)
    w_Whh = np.asarray(inputs["w_Whh"], f32)
    w_bih = np.asarray(inputs["w_bih"], f32)
    w_bhh = np.asarray(inputs["w_bhh"], f32)
    wa_W = np.asarray(inputs["wa_W"], f32)
    wa_b = np.asarray(inputs["wa_b"], f32)
    wa_v = np.asarray(inputs["wa_v"], f32)
    s_Wih = np.asarray(inputs["s_Wih"], f32)
    s_Whh = np.asarray(inputs["s_Whh"], f32)
    s_bih = np.asarray(inputs["s_bih"], f32)
    s_bhh = np.asarray(inputs["s_bhh"], f32)
    sa_W = np.asarray(inputs["sa_W"], f32)
    sa_b = np.asarray(inputs["sa_b"], f32)
    sa_v = np.asarray(inputs["sa_v"], f32)
    fc_W = np.asarray(inputs["fc_W"], f32)
    fc_b = np.asarray(inputs["fc_b"], f32)
    tokens = np.asarray(inputs["tokens"])

    def b(x):
        return np.ascontiguousarray(x.astype(bf16))

    # folded gather table G [V, 1536] = [rz0 | rz1 | n0 | n1]
    g0 = emb @ w_Wih[0].T + w_bih[0]
    g0[:, :512] += w_bhh[0][:512]
    g1 = emb @ w_Wih[1].T + w_bih[1]
    g1[:, :512] += w_bhh[1][:512]
    G = np.concatenate([g0[:, :512], g1[:, :512], g0[:, 512:], g1[:, 512:]], 1)

    whhT = np.stack([w_Whh[0].T[:128], w_Whh[0].T[128:],
                     w_Whh[1].T[:128], w_Whh[1].T[128:]])  # [4,128,768]
    brow = np.concatenate([w_bhh[0][512:], w_bhh[1][512:]])[None, :]
    vbc = np.broadcast_to(wa_v, (128, 512))

    # sentence input-proj table [512, 1536] with same col layout; bias row
    sg0 = s_Wih[0].T  # [512, 768]
    sg1 = s_Wih[1].T
    swihT = np.concatenate([sg0[:, :512], sg1[:, :512],
                            sg0[:, 512:], sg1[:, 512:]], 1)
    sprow = np.concatenate([
        s_bih[0][:512] + s_bhh[0][:512],
        s_bih[1][:512] + s_bhh[1][:512],
        s_bih[0][512:], s_bih[1][512:]])[None, :]
    swhhT = np.stack([s_Whh[0].T[:128], s_Whh[0].T[128:],
                      s_Whh[1].T[:128], s_Whh[1].T[128:]])
    # feature-major weight chunks for the sentence GRU, slot order
    # [d0r0 d0r1 d1r0 d1r1 d0z0 d0z1 d1z0 d1z1 d0n0 d0n1 d1n0 d1n1] x K
    goff = {"r": 0, "z": 256, "n": 512}
    fch = []
    for g in ("r", "z", "n"):
        for d in range(2):
            for m in range(2):
                for k in range(2):
                    fch.append(s_Whh[d][goff[g] + m * 128:goff[g] + (m + 1) * 128,
                                        k * 128:(k + 1) * 128].T)
    swhhF = np.stack(fch)  # [24, 128, 128]
    sbrow = np.concatenate([s_bhh[0][512:], s_bhh[1][512:]])[None, :]
    svbc = np.broadcast_to(sa_v, (128, 512))

    ind = np.zeros((128, 8), f32)
    for row in range(128):
        ind[row, row % 8] = 1.0

    shared = {
        "G": b(G), "whhT": b(whhT), "brow": b(brow),
        "waT": b(wa_W.T), "barow": b(wa_b[None, :]), "vb": b(vbc),
        "swihT": b(swihT), "sprow": b(sprow), "swhhT": b(swhhT),
        "swhhF": b(swhhF),
        "sbrowF": b(sbrow.reshape(4, 128)),
        "bones": b(np.repeat(np.eye(4, dtype=f32), 8, axis=1)),
        "sbrow": b(sbrow), "sawT": b(sa_W.T), "sbarow": b(sa_b[None, :]),
        "svb": b(svbc), "fcwT": b(fc_W.T), "fcb": b(fc_b[None, :]),
        "ind8": b(ind), "ind8f": np.ascontiguousarray(ind),
        "ind8T": np.ascontiguousarray(ind.T),
    }
    in_maps = []
    for c in range(NCORES):
        # word-row p = s*8 + doc  (so sentence step s owns partition rows
        # [s*8:(s+1)*8] of the batch-major sentence matrix)
        tk = np.ascontiguousarray(
            np.transpose(tokens[c * BC:(c + 1) * BC], (1, 0, 2))
            .reshape(NW, W).astype(np.int32))
        in_maps.append({**shared, "toks": tk})
    return in_maps


_NC_CACHE = {}


def _get_nc():
    if "nc" not in _NC_CACHE:
        _NC_CACHE["nc"] = _build_program()
    return _NC_CACHE["nc"]


def kernel(**inputs) -> np.ndarray:
    nc = _get_nc()
    in_maps = _prep_inputs(inputs)
    res = bass_utils.run_bass_kernel_spmd(nc, in_maps, core_ids=list(range(NCORES)))
    outs = []
    for c in range(NCORES):
        o = np.asarray(res.results[c]["out"], np.float32)
        outs.append(o)
    return np.concatenate(outs, 0)
